# revision 44
# baseline (speedup 1.0000x reference)
"""Trainium2 Bass kernel for CompositionalPINN forward.

Reference semantics (B=262144, H=256, N_STEPS=8):
    state = state_dz[:, :4]; qop = state_dz[:, 4:5]; dz_sub = state_dz[:, 5:6]/8
    n_full = floor(z_frac*8); frac = z_frac*8 - n_full
    for step in range(8):
        state += (n_full > step) * MLP(state, qop, dz_sub)        # residual MLP
    state += (frac > 1e-6) * MLP(state, qop, frac*dz_sub)
    MLP(x) = silu(silu(silu(x@W1+b1)@W2+b2)@W3+b3)@W4+b4  (6->256->256->256->4)

Strategy: pure data parallel over 8 cores.  Host transposes inputs to a
feature-major layout, precomputes the per-sample step masks, and sorts
samples by n_full (descending, dealt round-robin across cores) so each
512-sample tile only runs max(n_full)+1 MLP evals instead of 9.  The
per-tile eval schedule is baked into the compiled program.

Device schedule:
 - The scalar engine is the hard floor (864 silus of [128,1024],
   ~1.1us each, ~890us total); everything is arranged to keep it >94%
   busy.
 - Tiles are processed in PAIRS sharing SBUF columns: the even tile's
   x-rows live at partitions 0-7, the odd tile's at 64-71.  This lets
   the two K=8 L1 matmuls of a pair run CONCURRENTLY in different PE
   row strips (tile_position (0,0)/(64,0)) and the two M=4 L4 matmuls
   run concurrently in different column strips ((0,0)/(0,64)), hiding
   most of the small-matmul time.
 - Pair-evals flow through a STAGE-SKEWED software pipeline: 4 pair
   slots are admitted one column apart, so every emission column mixes
   one pair at L1, one at L2, one at L3 and one at L4.  The uniform
   PE/ACT load keeps the PE free of bursty idle windows that would
   re-throttle the HAM clock gate (which costs 2x PE speed), and keeps
   the scalar engine saturated.
 - L4 deltas are matmul'd into the just-consumed h3 psum tile (psum
   has_written is per-element), freeing both delta banks so the h psum
   pool gets 4 rotating [128,1024] tiles (all 8 banks).
 - bf16 weights/activations (fp32 state + psum): enables fast weight
   load and PE tile_position packing.
 - A short warm-up matmul burst + an early dummy silu (ACT table
   load) cover the initial input DMA.
"""

import numpy as np
import ml_dtypes
from contextlib import ExitStack

import concourse.bass as bass
import concourse.tile as tile
from concourse import bacc, mybir
from concourse.bass_utils import run_bass_kernel_spmd

F32 = mybir.dt.float32
BF16 = mybir.dt.bfloat16
Silu = mybir.ActivationFunctionType.Silu

NCORES = 8
NTILE = 512
CHUNK_TILES = 8                     # tiles per DMA chunk (= 4 pair-columns)
H = 256
NSTEPS = 8

# x row layout within an 8-row block (row 7 is a zero spare)
R_QOP = 4
R_DZSUB = 5
R_DZPART = 6

GROUP = 8                           # tiles interleaved per round (== chunk)
OPAR = 64                           # partition base of the odd pair member

WARMUP_MMS = 2                      # PE warm-up matmuls before real work


NSLOTS = 4                          # concurrently-in-flight pairs


def _pipeline_cols(schedule, tiles):
    """Software-pipeline column generator.  Four pair-slots run stage-skewed
    (admitted one per column), each cycling S0..S3 per eval.  Yields lists of
    (slot_idx, pair, eval_idx, stage).  Used by both the program builder and
    the host maskcat packer — must stay identical."""
    npairs = tiles // 2
    nev = [max(len(schedule[2 * q]), len(schedule[2 * q + 1]))
           for q in range(npairs)]
    next_pair = 0
    slots = [None] * NSLOTS
    while True:
        col = []
        admitted = False
        for i in range(NSLOTS):
            if slots[i] is None and next_pair < npairs and not admitted:
                slots[i] = [next_pair, 0, 0]
                next_pair += 1
                admitted = True
            s = slots[i]
            if s is None:
                continue
            q, ev, stage = s
            col.append((i, q, ev, stage))
            if stage == 3:
                if ev + 1 >= nev[q]:
                    slots[i] = None
                else:
                    s[1] = ev + 1
                    s[2] = 0
            else:
                s[2] = stage + 1
        if not col:
            break
        yield col


def _emit_order(schedule, tiles):
    """Yield (tile_index, eval_desc) in device MASK-CONSUMPTION order (the
    phase4 / stage-3 emissions of the pipeline)."""
    for col in _pipeline_cols(schedule, tiles):
        for (_, q, ev, stage) in col:
            if stage != 3:
                continue
            for t in (2 * q, 2 * q + 1):
                if ev < len(schedule[t]):
                    yield t, schedule[t][ev]


_BUILD_CACHE = {}

LAST_EXEC_NS = None  # set when BASSK_TRACE=1


def _install_ntff_hook():
    """The agent image lacks antenv.axon_hooks; synthesize it so
    run_bass_kernel_spmd(trace=True) can reach the NTFF profiler."""
    import sys
    import types
    if "antenv.axon_hooks" in sys.modules:
        return True
    try:
        import antenv
        from trn_agent_boot.trn_boot import _ntff_profile_via_ctypes
        hook = _ntff_profile_via_ctypes("/opt/axon/libaxon_pjrt.so")
        if hook is None:
            return False
        mod = types.ModuleType("antenv.axon_hooks")
        mod.get_axon_ntff_profile_hook = lambda: hook
        mod.set_axon_ntff_profile_hook = lambda h: None
        sys.modules["antenv.axon_hooks"] = mod
        antenv.axon_hooks = mod
        return True
    except Exception:
        return False


def _build(schedule, use_bias, n_core):
    """schedule: tuple over tiles of tuples of (is_partial, use_mask).

    Masked evals read consecutive NTILE-wide slots of the packed
    per-core mask tensor, in schedule order."""
    tiles = n_core // NTILE
    npair_cols = (tiles // 2) * NTILE          # SBUF columns (pair-major)
    n_masked = sum(1 for tev in schedule for (_, m) in tev if m)
    nm = max(1, n_masked)
    nc = bacc.Bacc("TRN2", target_bir_lowering=False, debug=False,
                   num_devices=NCORES)

    # xm dram rows 0-7: even tiles' 8 x-rows; rows 8-15: odd tiles'.
    xm_d = nc.declare_dram_parameter("xm", [16, npair_cols], F32, isOutput=False)
    mk_d = nc.declare_dram_parameter("maskcat", [8, nm * NTILE], F32,
                                     isOutput=False)
    w1_d = nc.declare_dram_parameter("w1", [8, 512], BF16, isOutput=False)
    w2_d = nc.declare_dram_parameter("w2", [128, 512], BF16, isOutput=False)
    w3_d = nc.declare_dram_parameter("w3", [128, 512], BF16, isOutput=False)
    w4_d = nc.declare_dram_parameter("w4", [128, 8], BF16, isOutput=False)
    if use_bias:
        b123_d = nc.declare_dram_parameter("b123", [128, 6], F32, isOutput=False)
        b4_d = nc.declare_dram_parameter("b4r", [8, 1], F32, isOutput=False)
    # out rows 0-3: even tiles' state; rows 4-7: odd tiles'.
    out_d = nc.declare_dram_parameter("outT", [8, npair_cols], F32, isOutput=True)

    chunks = [(c0, min(c0 + CHUNK_TILES, tiles))
              for c0 in range(0, tiles, CHUNK_TILES)]

    with tile.TileContext(nc) as tc, ExitStack() as ctx:
        const = ctx.enter_context(tc.tile_pool(name="const", bufs=1))
        data = ctx.enter_context(tc.tile_pool(name="data", bufs=1))
        stage = ctx.enter_context(tc.tile_pool(name="stage", bufs=4))
        acts = ctx.enter_context(tc.tile_pool(name="acts", bufs=12))
        tmp = ctx.enter_context(tc.tile_pool(name="tmp", bufs=3))
        ps_h = ctx.enter_context(tc.tile_pool(name="ps_h", bufs=4, space="PSUM"))

        # ---- weights (host-pre-rounded bf16).  w1 is replicated at
        # partitions 0-7 and 64-71 for the two L1 row strips.
        w1 = const.tile([OPAR + 8, 512], BF16)
        nc.gpsimd.dma_start(out=w1[0:8, :], in_=w1_d[:, :])
        nc.gpsimd.dma_start(out=w1[OPAR:OPAR + 8, :], in_=w1_d[:, :])
        w2 = const.tile([128, 512], BF16)
        nc.gpsimd.dma_start(out=w2, in_=w2_d[:, :])
        w3 = const.tile([128, 512], BF16)
        nc.gpsimd.dma_start(out=w3, in_=w3_d[:, :])
        w4 = const.tile([128, 8], BF16)
        nc.gpsimd.dma_start(out=w4, in_=w4_d[:, :])
        if use_bias:
            b123 = const.tile([128, 6], F32)
            nc.gpsimd.dma_start(out=b123, in_=b123_d[:, :])
            b4r = const.tile([OPAR + 4, 1], F32)
            nc.gpsimd.dma_start(out=b4r[0:4, :], in_=b4_d[0:4, :])
            nc.gpsimd.dma_start(out=b4r[OPAR:OPAR + 4, :], in_=b4_d[4:8, :])

        # ---- front-load the one-time silu ACT table load (~2.7us) while the
        # input DMA is still in flight (w1 is the first weight DMA to land)
        dummy = const.tile([8, 64], BF16)
        nc.scalar.activation(dummy, w1[0:8, 0:64], Silu)

        # ---- PE warm-up: back-to-back matmuls on the weight tiles keep the
        # PE busy (HAM stays un-throttled) while the xm DMA lands.
        warm = ps_h.tile([128, 2 * NTILE], F32, tag="h")
        for _ in range(WARMUP_MMS):
            nc.tensor.matmul(warm[:, 0:NTILE], w1[0:8, 0:128],
                             w1[0:8, 0:NTILE], start=True, stop=True)

        # ---- the full per-core dataset stays resident in SBUF; fp32 state
        # rows accumulate at full precision, each eval takes a rounded bf16
        # snapshot for the PE.
        xm = data.tile([OPAR + 8, npair_cols], F32)
        mkc = data.tile([OPAR + 4, nm * NTILE], F32)
        nc.gpsimd.dma_start(out=mkc[0:4, :], in_=mk_d[0:4, :])
        nc.gpsimd.dma_start(out=mkc[OPAR:OPAR + 4, :], in_=mk_d[4:8, :])
        mask_slot = [0]
        for (c0, c1) in chunks:
            cs = bass.ds(c0 * NTILE // 2, (c1 - c0) * NTILE // 2)
            nc.sync.dma_start(out=xm[0:8, cs], in_=xm_d[0:8, cs])
            nc.sync.dma_start(out=xm[OPAR:OPAR + 8, cs], in_=xm_d[8:16, cs])

        def phase1(q, members):
            # L1 for the pair: bf16 snapshot + up to 4 row-strip-concurrent
            # K=8 matmuls; silu per member.
            qs = bass.ds(q * NTILE, NTILE)
            xr = stage.tile([OPAR + 8, NTILE], BF16, tag="x16")
            for (t, base, is_partial, _) in members:
                nc.vector.tensor_copy(xr[base:base + 8, :], xm[base:base + 8, qs])
            hps = []
            for (t, base, is_partial, _) in members:
                w1off = 256 if is_partial else 0
                hp = ps_h.tile([128, 2 * NTILE], F32, tag="h")
                nc.tensor.matmul(hp[:, 0:NTILE],
                                 w1[base:base + 8, w1off:w1off + 128],
                                 xr[base:base + 8, :], start=True, stop=True,
                                 tile_position=(base, 0))
                nc.tensor.matmul(hp[:, NTILE:2 * NTILE],
                                 w1[base:base + 8, w1off + 128:w1off + 256],
                                 xr[base:base + 8, :], start=True, stop=True,
                                 tile_position=(base, 0))
                hps.append(hp)
            out = {}
            for (t, base, is_partial, _), hp in zip(members, hps):
                if use_bias:
                    nc.vector.tensor_scalar_add(hp[:, 0:NTILE], hp[:, 0:NTILE], b123[:, 0:1])
                    nc.vector.tensor_scalar_add(hp[:, NTILE:], hp[:, NTILE:], b123[:, 1:2])
                hs = acts.tile([128, 2 * NTILE], BF16, tag="h")
                nc.scalar.activation(hs, hp, Silu)
                out[t] = hs
            return out

        def phase_mid(t, w, hin, boff):
            # L2/L3: 4 matmuls (K=128 x2 accumulate, M=128 x2), silu
            hp = ps_h.tile([128, 2 * NTILE], F32, tag="h")
            for mt in range(2):
                for kt in range(2):
                    nc.tensor.matmul(
                        hp[:, mt * NTILE:(mt + 1) * NTILE],
                        w[:, kt * 256 + mt * 128: kt * 256 + (mt + 1) * 128],
                        hin[:, kt * NTILE:(kt + 1) * NTILE],
                        start=(kt == 0), stop=(kt == 1))
            if use_bias:
                nc.vector.tensor_scalar_add(hp[:, 0:NTILE], hp[:, 0:NTILE], b123[:, boff:boff + 1])
                nc.vector.tensor_scalar_add(hp[:, NTILE:], hp[:, NTILE:], b123[:, boff + 1:boff + 2])
            hs = acts.tile([128, 2 * NTILE], BF16, tag="h")
            nc.scalar.activation(hs, hp, Silu)
            return hp, hs

        def phase4(q, members, h3):
            # L4 for the pair: the member's delta is written back into its own
            # (already-consumed) h3 psum tile, rows base..base+4 — has_written
            # is per-element, so start=True over the stale bank is safe.  The
            # even member's rows 0-3 and the odd member's rows 64-67 go to
            # different column strips (concurrent).
            qs = bass.ds(q * NTILE, NTILE)
            for kt in range(2):
                for (t, base, _, _) in members:
                    hp3, hs3 = h3[t]
                    nc.tensor.matmul(hp3[base:base + 4, 0:NTILE],
                                     w4[:, 4 * kt:4 * kt + 4],
                                     hs3[:, kt * NTILE:(kt + 1) * NTILE],
                                     start=(kt == 0), stop=(kt == 1),
                                     tile_position=(0, base))
            for (t, base, _, use_mask) in members:
                db = h3[t][0][base:base + 4, 0:NTILE]
                if use_bias:
                    nc.vector.tensor_scalar_add(db, db, b4r[base:base + 4, 0:1])
                if not use_mask:
                    nc.vector.tensor_add(xm[base:base + 4, qs],
                                         xm[base:base + 4, qs], db)
                else:
                    j = mask_slot[0]
                    mask_slot[0] += 1
                    dm = tmp.tile([OPAR + 4, NTILE], F32, tag="dm")
                    nc.vector.tensor_mul(dm[base:base + 4, :], db,
                                         mkc[base:base + 4,
                                             j * NTILE:(j + 1) * NTILE])
                    nc.vector.tensor_add(xm[base:base + 4, qs],
                                         xm[base:base + 4, qs],
                                         dm[base:base + 4, :])

        # ---- stage-skewed software-pipeline emission: 4 pair-slots, each at
        # a different layer stage per column, for a uniform PE/ACT mix.
        def members_of(q, ev):
            mem = []
            if ev < len(schedule[2 * q]):
                mem.append((2 * q, 0, *schedule[2 * q][ev]))
            if ev < len(schedule[2 * q + 1]):
                mem.append((2 * q + 1, OPAR, *schedule[2 * q + 1][ev]))
            return mem

        ctxs = [{} for _ in range(NSLOTS)]
        for col in _pipeline_cols(schedule, tiles):
            for (i, q, ev, stg) in col:
                cx = ctxs[i]
                if stg == 0:
                    cx["mem"] = members_of(q, ev)
                    cx["h1"] = phase1(q, cx["mem"])
                elif stg == 1:
                    cx["h2"] = {t: phase_mid(t, w2, cx["h1"][t], 2)[1]
                                for (t, *_) in cx["mem"]}
                elif stg == 2:
                    cx["h3"] = {t: phase_mid(t, w3, cx["h2"][t], 4)
                                for (t, *_) in cx["mem"]}
                else:
                    phase4(q, cx["mem"], cx["h3"])

        # per-pair output DMAs: each fires as soon as its pair's final state
        # update lands (subtile deps), so only the last pair's 2 small DMAs
        # remain in the tail
        for q in range(tiles // 2):
            qs = bass.ds(q * NTILE, NTILE)
            nc.sync.dma_start(out=out_d[0:4, qs], in_=xm[0:4, qs])
            nc.sync.dma_start(out=out_d[4:8, qs], in_=xm[OPAR:OPAR + 4, qs])

    nc.compile()
    return nc


def kernel(state_dz, z_frac, W1, b1, W2, b2, W3, b3, W4, b4):
    global LAST_EXEC_NS
    import os

    state_dz = np.ascontiguousarray(state_dz, dtype=np.float32)
    z_frac = np.ascontiguousarray(z_frac, dtype=np.float32)
    W1 = np.asarray(W1, np.float32); W2 = np.asarray(W2, np.float32)
    W3 = np.asarray(W3, np.float32); W4 = np.asarray(W4, np.float32)
    b1 = np.asarray(b1, np.float32); b2 = np.asarray(b2, np.float32)
    b3 = np.asarray(b3, np.float32); b4 = np.asarray(b4, np.float32)

    B = state_dz.shape[0]
    assert B % (NCORES * 2 * NTILE) == 0, \
        f"B={B} must be divisible by {NCORES * 2 * NTILE}"
    n_core = B // NCORES
    tiles = n_core // NTILE

    # ---- host-side derived quantities (bitwise-identical fp32 ops vs jax)
    dz_sub = (state_dz[:, 5] / np.float32(8.0)).astype(np.float32)
    cont = (z_frac * np.float32(NSTEPS)).astype(np.float32)
    n_full = np.floor(cont).astype(np.float32)
    frac = (cont - n_full).astype(np.float32)
    dz_part = (frac * dz_sub).astype(np.float32)
    has_part = (frac > np.float32(1e-6)).astype(np.float32)
    n_int = np.minimum(n_full, NSTEPS).astype(np.int64)

    # ---- sort desc by n_full, deal round-robin to cores
    order = np.argsort(-n_int, kind="stable")
    perms = [order[c::NCORES] for c in range(NCORES)]

    # ---- build per-core xm arrays [16, npair_cols]: rows 0-7 even tiles,
    # rows 8-15 odd tiles, pair q at columns [q*NTILE:(q+1)*NTILE]
    npair_cols = (tiles // 2) * NTILE
    xms = []
    for c in range(NCORES):
        p = perms[c]
        x8 = np.zeros((8, n_core), np.float32)
        x8[0:4] = state_dz[p, 0:4].T
        x8[R_QOP] = state_dz[p, 4]
        x8[R_DZSUB] = dz_sub[p]
        x8[R_DZPART] = dz_part[p]
        xg = np.zeros((16, npair_cols), np.float32)
        x8t = x8.reshape(8, tiles, NTILE)
        xg = np.concatenate([x8t[:, 0::2, :], x8t[:, 1::2, :]],
                            axis=0).reshape(16, npair_cols)
        xms.append(np.ascontiguousarray(xg))

    # ---- union schedule across cores (SPMD: one program for all cores)
    sched = []
    for t in range(tiles):
        sl = slice(t * NTILE, (t + 1) * NTILE)
        smax, smin = 0, NSTEPS
        anyp, allp = False, True
        for c in range(NCORES):
            nf = n_int[perms[c][sl]]
            smax = max(smax, int(nf.max()))
            smin = min(smin, int(nf.min()))
            hp = has_part[perms[c][sl]]
            anyp = anyp or bool(hp.any())
            allp = allp and bool(hp.all())
        evals = []
        for s in range(min(smax, NSTEPS)):
            evals.append((False, smin <= s))
        if anyp:
            evals.append((True, not allp))
        sched.append(tuple(evals))
    sched = tuple(sched)

    # masked evals in DEVICE EMISSION order; track each tile's step counter
    masked_evals = []
    step_no = [0] * tiles
    for t, (is_partial, use_mask) in _emit_order(sched, tiles):
        s = None if is_partial else step_no[t]
        if not is_partial:
            step_no[t] += 1
        if use_mask:
            masked_evals.append((t, s))

    # ---- packed mask rows, one NTILE slot per masked eval, per core.
    # Dram rows 0-3 serve even tiles, rows 4-7 odd tiles.
    nm = max(1, len(masked_evals))
    maskcats = [np.zeros((8, nm * NTILE), np.float32) for _ in range(NCORES)]
    for j, (t, s) in enumerate(masked_evals):
        sl = slice(t * NTILE, (t + 1) * NTILE)
        r0 = 0 if t % 2 == 0 else 4
        for c in range(NCORES):
            idx = perms[c][sl]
            row = has_part[idx] if s is None else (n_full[idx] > s).astype(np.float32)
            maskcats[c][r0:r0 + 4, j * NTILE:(j + 1) * NTILE] = row[None, :]

    use_bias = bool(np.any(b1) or np.any(b2) or np.any(b3) or np.any(b4))

    key = (sched, use_bias, n_core)
    if key not in _BUILD_CACHE:
        _BUILD_CACHE[key] = _build(sched, use_bias, n_core)
    nc = _BUILD_CACHE[key]

    # ---- weight tensors in lhsT layouts (bf16)
    w1h = np.zeros((8, 512), np.float32)
    w1h[0:6, 0:256] = W1                      # full: state,qop,dz_sub
    w1h[0:5, 256:512] = W1[0:5]               # partial: dz slot zeroed,
    w1h[6, 256:512] = W1[5]                   # dz weight reads dz_partial row
    w1h = w1h.astype(ml_dtypes.bfloat16)
    w2h = np.concatenate([W2[0:128], W2[128:256]], axis=1).astype(ml_dtypes.bfloat16)
    w3h = np.concatenate([W3[0:128], W3[128:256]], axis=1).astype(ml_dtypes.bfloat16)
    w4h = np.concatenate([W4[0:128], W4[128:256]], axis=1).astype(ml_dtypes.bfloat16)

    in_map = {"w1": w1h, "w2": w2h, "w3": w3h, "w4": w4h}
    if use_bias:
        b123 = np.stack([b1[0:128], b1[128:256], b2[0:128], b2[128:256],
                         b3[0:128], b3[128:256]], axis=1).astype(np.float32)
        in_map["b123"] = b123
        in_map["b4r"] = np.concatenate([b4, b4]).reshape(8, 1).astype(np.float32)

    in_maps = [{**in_map, "xm": xms[c], "maskcat": maskcats[c]}
               for c in range(NCORES)]

    trace = os.environ.get("BASSK_TRACE") == "1" and _install_ntff_hook()
    try:
        res = run_bass_kernel_spmd(nc, in_maps, list(range(NCORES)), trace=trace)
    except Exception:
        if not trace:
            raise
        res = run_bass_kernel_spmd(nc, in_maps, list(range(NCORES)), trace=False)
    LAST_EXEC_NS = res.exec_time_ns

    out = np.empty((B, 4), np.float32)
    for c in range(NCORES):
        og = res.results[c]["outT"]                       # [8, npair_cols]
        oc = np.empty((4, n_core), np.float32)
        ocv = oc.reshape(4, tiles, NTILE)
        ogv = og.reshape(8, tiles // 2, NTILE)
        ocv[:, 0::2, :] = ogv[0:4]
        ocv[:, 1::2, :] = ogv[4:8]
        out[perms[c], :] = oc.T
    return out


# revision 46
# speedup vs baseline: 1.0075x; 1.0075x over previous
"""Trainium2 Bass kernel for CompositionalPINN forward.

Reference semantics (B=262144, H=256, N_STEPS=8):
    state = state_dz[:, :4]; qop = state_dz[:, 4:5]; dz_sub = state_dz[:, 5:6]/8
    n_full = floor(z_frac*8); frac = z_frac*8 - n_full
    for step in range(8):
        state += (n_full > step) * MLP(state, qop, dz_sub)        # residual MLP
    state += (frac > 1e-6) * MLP(state, qop, frac*dz_sub)
    MLP(x) = silu(silu(silu(x@W1+b1)@W2+b2)@W3+b3)@W4+b4  (6->256->256->256->4)

Strategy: pure data parallel over 8 cores.  Host transposes inputs to a
feature-major layout, precomputes the per-sample step masks, and sorts
samples by n_full (descending, dealt round-robin across cores) so each
512-sample tile only runs max(n_full)+1 MLP evals instead of 9.  The
per-tile eval schedule is baked into the compiled program.

Device schedule:
 - The scalar engine is the hard floor (864 silus of [128,1024],
   ~1.1us each, ~890us total); everything is arranged to keep it >94%
   busy.
 - Tiles are processed in PAIRS sharing SBUF columns: the even tile's
   x-rows live at partitions 0-7, the odd tile's at 64-71.  This lets
   the two K=8 L1 matmuls of a pair run CONCURRENTLY in different PE
   row strips (tile_position (0,0)/(64,0)) and the two M=4 L4 matmuls
   run concurrently in different column strips ((0,0)/(0,64)), hiding
   most of the small-matmul time.
 - Pair-evals flow through a STAGE-SKEWED software pipeline: 4 pair
   slots are admitted one column apart, so every emission column mixes
   one pair at L1, one at L2, one at L3 and one at L4.  The uniform
   PE/ACT load keeps the PE free of bursty idle windows that would
   re-throttle the HAM clock gate (which costs 2x PE speed), and keeps
   the scalar engine saturated.
 - L4 deltas are matmul'd into the just-consumed h3 psum tile (psum
   has_written is per-element), freeing both delta banks so the h psum
   pool gets 4 rotating [128,1024] tiles (all 8 banks).
 - bf16 weights/activations (fp32 state + psum): enables fast weight
   load and PE tile_position packing.
 - A short warm-up matmul burst + an early dummy silu (ACT table
   load) cover the initial input DMA.
"""

import numpy as np
import ml_dtypes
from contextlib import ExitStack

import concourse.bass as bass
import concourse.tile as tile
from concourse import bacc, mybir
from concourse.bass_utils import run_bass_kernel_spmd

F32 = mybir.dt.float32
BF16 = mybir.dt.bfloat16
Silu = mybir.ActivationFunctionType.Silu

NCORES = 8
NTILE = 512
CHUNK_TILES = 8                     # tiles per DMA chunk (= 4 pair-columns)
H = 256
NSTEPS = 8

# x row layout within an 8-row block (row 7 is a zero spare)
R_QOP = 4
R_DZSUB = 5
R_DZPART = 6

GROUP = 8                           # tiles interleaved per round (== chunk)
OPAR = 64                           # partition base of the odd pair member

WARMUP_MMS = 2                      # PE warm-up matmuls before real work


NSLOTS = 4                          # concurrently-in-flight pairs


def _pipeline_cols(schedule, tiles):
    """Software-pipeline column generator.  Four pair-slots run stage-skewed
    (admitted one per column), each cycling S0..S3 per eval.  Yields lists of
    (slot_idx, pair, eval_idx, stage).  Used by both the program builder and
    the host maskcat packer — must stay identical."""
    npairs = tiles // 2
    nev = [max(len(schedule[2 * q]), len(schedule[2 * q + 1]))
           for q in range(npairs)]
    next_pair = 0
    slots = [None] * NSLOTS
    while True:
        col = []
        admitted = False
        for i in range(NSLOTS):
            if slots[i] is None and next_pair < npairs and not admitted:
                slots[i] = [next_pair, 0, 0]
                next_pair += 1
                admitted = True
            s = slots[i]
            if s is None:
                continue
            q, ev, stage = s
            col.append((i, q, ev, stage))
            if stage == 3:
                if ev + 1 >= nev[q]:
                    slots[i] = None
                else:
                    s[1] = ev + 1
                    s[2] = 0
            else:
                s[2] = stage + 1
        if not col:
            break
        yield col


def _emit_order(schedule, tiles):
    """Yield (tile_index, eval_desc) in device MASK-CONSUMPTION order (the
    phase4 / stage-3 emissions of the pipeline)."""
    for col in _pipeline_cols(schedule, tiles):
        for (_, q, ev, stage) in col:
            if stage != 3:
                continue
            for t in (2 * q, 2 * q + 1):
                if ev < len(schedule[t]):
                    yield t, schedule[t][ev]


_BUILD_CACHE = {}

LAST_EXEC_NS = None  # set when BASSK_TRACE=1


def _install_ntff_hook():
    """The agent image lacks antenv.axon_hooks; synthesize it so
    run_bass_kernel_spmd(trace=True) can reach the NTFF profiler."""
    import sys
    import types
    if "antenv.axon_hooks" in sys.modules:
        return True
    try:
        import antenv
        from trn_agent_boot.trn_boot import _ntff_profile_via_ctypes
        hook = _ntff_profile_via_ctypes("/opt/axon/libaxon_pjrt.so")
        if hook is None:
            return False
        mod = types.ModuleType("antenv.axon_hooks")
        mod.get_axon_ntff_profile_hook = lambda: hook
        mod.set_axon_ntff_profile_hook = lambda h: None
        sys.modules["antenv.axon_hooks"] = mod
        antenv.axon_hooks = mod
        return True
    except Exception:
        return False


def _build(schedule, use_bias, n_core):
    """schedule: tuple over tiles of tuples of (is_partial, use_mask).

    Masked evals read consecutive NTILE-wide slots of the packed
    per-core mask tensor, in schedule order."""
    tiles = n_core // NTILE
    npair_cols = (tiles // 2) * NTILE          # SBUF columns (pair-major)
    n_masked = sum(1 for tev in schedule for (_, m) in tev if m)
    nm = max(1, n_masked)
    nc = bacc.Bacc("TRN2", target_bir_lowering=False, debug=False,
                   num_devices=NCORES)

    # xm dram rows 0-7: even tiles' 8 x-rows; rows 8-15: odd tiles'.
    xm_d = nc.declare_dram_parameter("xm", [16, npair_cols], F32, isOutput=False)
    mk_d = nc.declare_dram_parameter("maskcat", [8, nm * NTILE], F32,
                                     isOutput=False)
    w1_d = nc.declare_dram_parameter("w1", [8, 512], BF16, isOutput=False)
    w2_d = nc.declare_dram_parameter("w2", [128, 512], BF16, isOutput=False)
    w3_d = nc.declare_dram_parameter("w3", [128, 512], BF16, isOutput=False)
    w4_d = nc.declare_dram_parameter("w4", [128, 8], BF16, isOutput=False)
    if use_bias:
        b123_d = nc.declare_dram_parameter("b123", [128, 6], F32, isOutput=False)
        b4_d = nc.declare_dram_parameter("b4r", [8, 1], F32, isOutput=False)
    # out rows 0-3: even tiles' state; rows 4-7: odd tiles'.
    out_d = nc.declare_dram_parameter("outT", [8, npair_cols], F32, isOutput=True)

    chunks = [(c0, min(c0 + CHUNK_TILES, tiles))
              for c0 in range(0, tiles, CHUNK_TILES)]

    with tile.TileContext(nc) as tc, ExitStack() as ctx:
        const = ctx.enter_context(tc.tile_pool(name="const", bufs=1))
        data = ctx.enter_context(tc.tile_pool(name="data", bufs=1))
        stage = ctx.enter_context(tc.tile_pool(name="stage", bufs=3))
        acts = ctx.enter_context(tc.tile_pool(name="acts", bufs=10))
        tmp = ctx.enter_context(tc.tile_pool(name="tmp", bufs=2))
        ps_h = ctx.enter_context(tc.tile_pool(name="ps_h", bufs=4, space="PSUM"))

        # ---- weights (host-pre-rounded bf16).  w1 is replicated at
        # partitions 0-7 and 64-71 for the two L1 row strips.
        w1 = const.tile([OPAR + 8, 512], BF16)
        nc.gpsimd.dma_start(out=w1[0:8, :], in_=w1_d[:, :])
        nc.gpsimd.dma_start(out=w1[OPAR:OPAR + 8, :], in_=w1_d[:, :])
        w2 = const.tile([128, 512], BF16)
        nc.gpsimd.dma_start(out=w2, in_=w2_d[:, :])
        w3 = const.tile([128, 512], BF16)
        nc.gpsimd.dma_start(out=w3, in_=w3_d[:, :])
        w4 = const.tile([128, 8], BF16)
        nc.gpsimd.dma_start(out=w4, in_=w4_d[:, :])
        if use_bias:
            b123 = const.tile([128, 6], F32)
            nc.gpsimd.dma_start(out=b123, in_=b123_d[:, :])
            b4r = const.tile([OPAR + 4, 1], F32)
            nc.gpsimd.dma_start(out=b4r[0:4, :], in_=b4_d[0:4, :])
            nc.gpsimd.dma_start(out=b4r[OPAR:OPAR + 4, :], in_=b4_d[4:8, :])

        # ---- front-load the one-time silu ACT table load (~2.7us) while the
        # input DMA is still in flight (w1 is the first weight DMA to land)
        dummy = const.tile([8, 64], BF16)
        nc.scalar.activation(dummy, w1[0:8, 0:64], Silu)

        # ---- PE warm-up: back-to-back matmuls on the weight tiles keep the
        # PE busy (HAM stays un-throttled) while the xm DMA lands.
        warm = ps_h.tile([128, 2 * NTILE], F32, tag="h")
        for _ in range(WARMUP_MMS):
            nc.tensor.matmul(warm[:, 0:NTILE], w1[0:8, 0:128],
                             w1[0:8, 0:NTILE], start=True, stop=True)

        # ---- the full per-core dataset stays resident in SBUF; fp32 state
        # rows accumulate at full precision, each eval takes a rounded bf16
        # snapshot for the PE.
        xm = data.tile([OPAR + 8, npair_cols], F32)
        mkc = data.tile([OPAR + 4, nm * NTILE], F32)
        nc.gpsimd.dma_start(out=mkc[0:4, :], in_=mk_d[0:4, :])
        nc.gpsimd.dma_start(out=mkc[OPAR:OPAR + 4, :], in_=mk_d[4:8, :])
        mask_slot = [0]
        for (c0, c1) in chunks:
            cs = bass.ds(c0 * NTILE // 2, (c1 - c0) * NTILE // 2)
            nc.sync.dma_start(out=xm[0:8, cs], in_=xm_d[0:8, cs])
            nc.sync.dma_start(out=xm[OPAR:OPAR + 8, cs], in_=xm_d[8:16, cs])

        def phase1(q, members):
            # L1 for the pair: bf16 snapshot + up to 4 row-strip-concurrent
            # K=8 matmuls; silu per member.
            qs = bass.ds(q * NTILE, NTILE)
            xr = stage.tile([OPAR + 8, NTILE], BF16, tag="x16")
            if len(members) == 2:
                # one DVE op spanning both members' partition homes (the rows
                # in between are unused; copying them is free — DVE cost is
                # free-dim only)
                nc.vector.tensor_copy(xr[0:OPAR + 8, :], xm[0:OPAR + 8, qs])
            else:
                base = members[0][1]
                nc.vector.tensor_copy(xr[base:base + 8, :], xm[base:base + 8, qs])
            hps = []
            for (t, base, is_partial, _) in members:
                w1off = 256 if is_partial else 0
                hp = ps_h.tile([128, 2 * NTILE], F32, tag="h")
                nc.tensor.matmul(hp[:, 0:NTILE],
                                 w1[base:base + 8, w1off:w1off + 128],
                                 xr[base:base + 8, :], start=True, stop=True,
                                 tile_position=(base, 0))
                nc.tensor.matmul(hp[:, NTILE:2 * NTILE],
                                 w1[base:base + 8, w1off + 128:w1off + 256],
                                 xr[base:base + 8, :], start=True, stop=True,
                                 tile_position=(base, 0))
                hps.append(hp)
            out = {}
            for (t, base, is_partial, _), hp in zip(members, hps):
                if use_bias:
                    nc.vector.tensor_scalar_add(hp[:, 0:NTILE], hp[:, 0:NTILE], b123[:, 0:1])
                    nc.vector.tensor_scalar_add(hp[:, NTILE:], hp[:, NTILE:], b123[:, 1:2])
                hs = acts.tile([128, 2 * NTILE], BF16, tag="h")
                nc.scalar.activation(hs, hp, Silu)
                out[t] = hs
            return out

        def phase_mid(t, w, hin, boff):
            # L2/L3: 4 matmuls (K=128 x2 accumulate, M=128 x2), silu
            hp = ps_h.tile([128, 2 * NTILE], F32, tag="h")
            for mt in range(2):
                for kt in range(2):
                    nc.tensor.matmul(
                        hp[:, mt * NTILE:(mt + 1) * NTILE],
                        w[:, kt * 256 + mt * 128: kt * 256 + (mt + 1) * 128],
                        hin[:, kt * NTILE:(kt + 1) * NTILE],
                        start=(kt == 0), stop=(kt == 1))
            if use_bias:
                nc.vector.tensor_scalar_add(hp[:, 0:NTILE], hp[:, 0:NTILE], b123[:, boff:boff + 1])
                nc.vector.tensor_scalar_add(hp[:, NTILE:], hp[:, NTILE:], b123[:, boff + 1:boff + 2])
            hs = acts.tile([128, 2 * NTILE], BF16, tag="h")
            nc.scalar.activation(hs, hp, Silu)
            return hp, hs

        def phase4(q, members, h3):
            # L4 for the pair: the member's delta is written back into its own
            # (already-consumed) h3 psum tile, rows base..base+4 — has_written
            # is per-element, so start=True over the stale bank is safe.  The
            # even member's rows 0-3 and the odd member's rows 64-67 go to
            # different column strips (concurrent).
            qs = bass.ds(q * NTILE, NTILE)
            for kt in range(2):
                for (t, base, _, _) in members:
                    hp3, hs3 = h3[t]
                    nc.tensor.matmul(hp3[base:base + 4, 0:NTILE],
                                     w4[:, 4 * kt:4 * kt + 4],
                                     hs3[:, kt * NTILE:(kt + 1) * NTILE],
                                     start=(kt == 0), stop=(kt == 1),
                                     tile_position=(0, base))
            for (t, base, _, use_mask) in members:
                db = h3[t][0][base:base + 4, 0:NTILE]
                if use_bias:
                    nc.vector.tensor_scalar_add(db, db, b4r[base:base + 4, 0:1])
                if not use_mask:
                    nc.vector.tensor_add(xm[base:base + 4, qs],
                                         xm[base:base + 4, qs], db)
                else:
                    j = mask_slot[0]
                    mask_slot[0] += 1
                    dm = tmp.tile([OPAR + 4, NTILE], F32, tag="dm")
                    nc.vector.tensor_mul(dm[base:base + 4, :], db,
                                         mkc[base:base + 4,
                                             j * NTILE:(j + 1) * NTILE])
                    nc.vector.tensor_add(xm[base:base + 4, qs],
                                         xm[base:base + 4, qs],
                                         dm[base:base + 4, :])

        # ---- stage-skewed software-pipeline emission: 4 pair-slots, each at
        # a different layer stage per column, for a uniform PE/ACT mix.
        def members_of(q, ev):
            mem = []
            if ev < len(schedule[2 * q]):
                mem.append((2 * q, 0, *schedule[2 * q][ev]))
            if ev < len(schedule[2 * q + 1]):
                mem.append((2 * q + 1, OPAR, *schedule[2 * q + 1][ev]))
            return mem

        ctxs = [{} for _ in range(NSLOTS)]
        for col in _pipeline_cols(schedule, tiles):
            for (i, q, ev, stg) in col:
                cx = ctxs[i]
                if stg == 0:
                    cx["mem"] = members_of(q, ev)
                    cx["h1"] = phase1(q, cx["mem"])
                elif stg == 1:
                    cx["h2"] = {t: phase_mid(t, w2, cx["h1"][t], 2)[1]
                                for (t, *_) in cx["mem"]}
                elif stg == 2:
                    cx["h3"] = {t: phase_mid(t, w3, cx["h2"][t], 4)
                                for (t, *_) in cx["mem"]}
                else:
                    phase4(q, cx["mem"], cx["h3"])

        # per-pair output DMAs: each fires as soon as its pair's final state
        # update lands (subtile deps), so only the last pair's 2 small DMAs
        # remain in the tail
        for q in range(tiles // 2):
            qs = bass.ds(q * NTILE, NTILE)
            nc.sync.dma_start(out=out_d[0:4, qs], in_=xm[0:4, qs])
            nc.sync.dma_start(out=out_d[4:8, qs], in_=xm[OPAR:OPAR + 4, qs])

    nc.compile()
    return nc


def kernel(state_dz, z_frac, W1, b1, W2, b2, W3, b3, W4, b4):
    global LAST_EXEC_NS
    import os

    state_dz = np.ascontiguousarray(state_dz, dtype=np.float32)
    z_frac = np.ascontiguousarray(z_frac, dtype=np.float32)
    W1 = np.asarray(W1, np.float32); W2 = np.asarray(W2, np.float32)
    W3 = np.asarray(W3, np.float32); W4 = np.asarray(W4, np.float32)
    b1 = np.asarray(b1, np.float32); b2 = np.asarray(b2, np.float32)
    b3 = np.asarray(b3, np.float32); b4 = np.asarray(b4, np.float32)

    B = state_dz.shape[0]
    assert B % (NCORES * 2 * NTILE) == 0, \
        f"B={B} must be divisible by {NCORES * 2 * NTILE}"
    n_core = B // NCORES
    tiles = n_core // NTILE

    # ---- host-side derived quantities (bitwise-identical fp32 ops vs jax)
    dz_sub = (state_dz[:, 5] / np.float32(8.0)).astype(np.float32)
    cont = (z_frac * np.float32(NSTEPS)).astype(np.float32)
    n_full = np.floor(cont).astype(np.float32)
    frac = (cont - n_full).astype(np.float32)
    dz_part = (frac * dz_sub).astype(np.float32)
    has_part = (frac > np.float32(1e-6)).astype(np.float32)
    n_int = np.minimum(n_full, NSTEPS).astype(np.int64)

    # ---- sort desc by n_full, deal round-robin to cores
    order = np.argsort(-n_int, kind="stable")
    perms = [order[c::NCORES] for c in range(NCORES)]

    # ---- build per-core xm arrays [16, npair_cols]: rows 0-7 even tiles,
    # rows 8-15 odd tiles, pair q at columns [q*NTILE:(q+1)*NTILE]
    npair_cols = (tiles // 2) * NTILE
    xms = []
    for c in range(NCORES):
        p = perms[c]
        x8 = np.zeros((8, n_core), np.float32)
        x8[0:4] = state_dz[p, 0:4].T
        x8[R_QOP] = state_dz[p, 4]
        x8[R_DZSUB] = dz_sub[p]
        x8[R_DZPART] = dz_part[p]
        xg = np.zeros((16, npair_cols), np.float32)
        x8t = x8.reshape(8, tiles, NTILE)
        xg = np.concatenate([x8t[:, 0::2, :], x8t[:, 1::2, :]],
                            axis=0).reshape(16, npair_cols)
        xms.append(np.ascontiguousarray(xg))

    # ---- union schedule across cores (SPMD: one program for all cores)
    sched = []
    for t in range(tiles):
        sl = slice(t * NTILE, (t + 1) * NTILE)
        smax, smin = 0, NSTEPS
        anyp, allp = False, True
        for c in range(NCORES):
            nf = n_int[perms[c][sl]]
            smax = max(smax, int(nf.max()))
            smin = min(smin, int(nf.min()))
            hp = has_part[perms[c][sl]]
            anyp = anyp or bool(hp.any())
            allp = allp and bool(hp.all())
        evals = []
        for s in range(min(smax, NSTEPS)):
            evals.append((False, smin <= s))
        if anyp:
            evals.append((True, not allp))
        sched.append(tuple(evals))
    sched = tuple(sched)

    # masked evals in DEVICE EMISSION order; track each tile's step counter
    masked_evals = []
    step_no = [0] * tiles
    for t, (is_partial, use_mask) in _emit_order(sched, tiles):
        s = None if is_partial else step_no[t]
        if not is_partial:
            step_no[t] += 1
        if use_mask:
            masked_evals.append((t, s))

    # ---- packed mask rows, one NTILE slot per masked eval, per core.
    # Dram rows 0-3 serve even tiles, rows 4-7 odd tiles.
    nm = max(1, len(masked_evals))
    maskcats = [np.zeros((8, nm * NTILE), np.float32) for _ in range(NCORES)]
    for j, (t, s) in enumerate(masked_evals):
        sl = slice(t * NTILE, (t + 1) * NTILE)
        r0 = 0 if t % 2 == 0 else 4
        for c in range(NCORES):
            idx = perms[c][sl]
            row = has_part[idx] if s is None else (n_full[idx] > s).astype(np.float32)
            maskcats[c][r0:r0 + 4, j * NTILE:(j + 1) * NTILE] = row[None, :]

    use_bias = bool(np.any(b1) or np.any(b2) or np.any(b3) or np.any(b4))

    key = (sched, use_bias, n_core)
    if key not in _BUILD_CACHE:
        _BUILD_CACHE[key] = _build(sched, use_bias, n_core)
    nc = _BUILD_CACHE[key]

    # ---- weight tensors in lhsT layouts (bf16)
    w1h = np.zeros((8, 512), np.float32)
    w1h[0:6, 0:256] = W1                      # full: state,qop,dz_sub
    w1h[0:5, 256:512] = W1[0:5]               # partial: dz slot zeroed,
    w1h[6, 256:512] = W1[5]                   # dz weight reads dz_partial row
    w1h = w1h.astype(ml_dtypes.bfloat16)
    w2h = np.concatenate([W2[0:128], W2[128:256]], axis=1).astype(ml_dtypes.bfloat16)
    w3h = np.concatenate([W3[0:128], W3[128:256]], axis=1).astype(ml_dtypes.bfloat16)
    w4h = np.concatenate([W4[0:128], W4[128:256]], axis=1).astype(ml_dtypes.bfloat16)

    in_map = {"w1": w1h, "w2": w2h, "w3": w3h, "w4": w4h}
    if use_bias:
        b123 = np.stack([b1[0:128], b1[128:256], b2[0:128], b2[128:256],
                         b3[0:128], b3[128:256]], axis=1).astype(np.float32)
        in_map["b123"] = b123
        in_map["b4r"] = np.concatenate([b4, b4]).reshape(8, 1).astype(np.float32)

    in_maps = [{**in_map, "xm": xms[c], "maskcat": maskcats[c]}
               for c in range(NCORES)]

    trace = os.environ.get("BASSK_TRACE") == "1" and _install_ntff_hook()
    try:
        res = run_bass_kernel_spmd(nc, in_maps, list(range(NCORES)), trace=trace)
    except Exception:
        if not trace:
            raise
        res = run_bass_kernel_spmd(nc, in_maps, list(range(NCORES)), trace=False)
    LAST_EXEC_NS = res.exec_time_ns

    out = np.empty((B, 4), np.float32)
    for c in range(NCORES):
        og = res.results[c]["outT"]                       # [8, npair_cols]
        oc = np.empty((4, n_core), np.float32)
        ocv = oc.reshape(4, tiles, NTILE)
        ogv = og.reshape(8, tiles // 2, NTILE)
        ocv[:, 0::2, :] = ogv[0:4]
        ocv[:, 1::2, :] = ogv[4:8]
        out[perms[c], :] = oc.T
    return out


# revision 47
# speedup vs baseline: 1.0084x; 1.0009x over previous
"""Trainium2 Bass kernel for CompositionalPINN forward.

Reference semantics (B=262144, H=256, N_STEPS=8):
    state = state_dz[:, :4]; qop = state_dz[:, 4:5]; dz_sub = state_dz[:, 5:6]/8
    n_full = floor(z_frac*8); frac = z_frac*8 - n_full
    for step in range(8):
        state += (n_full > step) * MLP(state, qop, dz_sub)        # residual MLP
    state += (frac > 1e-6) * MLP(state, qop, frac*dz_sub)
    MLP(x) = silu(silu(silu(x@W1+b1)@W2+b2)@W3+b3)@W4+b4  (6->256->256->256->4)

Strategy: pure data parallel over 8 cores.  Host transposes inputs to a
feature-major layout, precomputes the per-sample step masks, and sorts
samples by n_full (descending, dealt round-robin across cores) so each
512-sample tile only runs max(n_full)+1 MLP evals instead of 9.  The
per-tile eval schedule is baked into the compiled program.

Device schedule:
 - The scalar engine is the hard floor (864 silus of [128,1024],
   ~1.1us each, ~890us total); everything is arranged to keep it >94%
   busy.
 - Tiles are processed in PAIRS sharing SBUF columns: the even tile's
   x-rows live at partitions 0-7, the odd tile's at 64-71.  This lets
   the two K=8 L1 matmuls of a pair run CONCURRENTLY in different PE
   row strips (tile_position (0,0)/(64,0)) and the two M=4 L4 matmuls
   run concurrently in different column strips ((0,0)/(0,64)), hiding
   most of the small-matmul time.
 - Pair-evals flow through a STAGE-SKEWED software pipeline: 4 pair
   slots are admitted one column apart, so every emission column mixes
   one pair at L1, one at L2, one at L3 and one at L4.  The uniform
   PE/ACT load keeps the PE free of bursty idle windows that would
   re-throttle the HAM clock gate (which costs 2x PE speed), and keeps
   the scalar engine saturated.
 - L4 deltas are matmul'd into the just-consumed h3 psum tile (psum
   has_written is per-element), freeing both delta banks so the h psum
   pool gets 4 rotating [128,1024] tiles (all 8 banks).
 - bf16 weights/activations (fp32 state + psum): enables fast weight
   load and PE tile_position packing.
 - A short warm-up matmul burst + an early dummy silu (ACT table
   load) cover the initial input DMA.
"""

import numpy as np
import ml_dtypes
from contextlib import ExitStack

import concourse.bass as bass
import concourse.tile as tile
from concourse import bacc, mybir
from concourse.bass_utils import run_bass_kernel_spmd

F32 = mybir.dt.float32
BF16 = mybir.dt.bfloat16
Silu = mybir.ActivationFunctionType.Silu

NCORES = 8
NTILE = 512
CHUNK_TILES = 8                     # tiles per DMA chunk (= 4 pair-columns)
H = 256
NSTEPS = 8

# x row layout within an 8-row block (row 7 is a zero spare)
R_QOP = 4
R_DZSUB = 5
R_DZPART = 6

GROUP = 8                           # tiles interleaved per round (== chunk)
OPAR = 64                           # partition base of the odd pair member

WARMUP_MMS = 2                      # PE warm-up matmuls before real work


NSLOTS = 4                          # concurrently-in-flight pairs


def _pipeline_cols(schedule, tiles):
    """Software-pipeline column generator.  Four pair-slots run stage-skewed
    (admitted one per column), each cycling S0..S3 per eval.  Yields lists of
    (slot_idx, pair, eval_idx, stage).  Used by both the program builder and
    the host maskcat packer — must stay identical."""
    npairs = tiles // 2
    nev = [max(len(schedule[2 * q]), len(schedule[2 * q + 1]))
           for q in range(npairs)]
    next_pair = 0
    slots = [None] * NSLOTS
    while True:
        col = []
        admitted = False
        for i in range(NSLOTS):
            if slots[i] is None and next_pair < npairs and not admitted:
                slots[i] = [next_pair, 0, 0]
                next_pair += 1
                admitted = True
            s = slots[i]
            if s is None:
                continue
            q, ev, stage = s
            col.append((i, q, ev, stage))
            if stage == 3:
                if ev + 1 >= nev[q]:
                    slots[i] = None
                else:
                    s[1] = ev + 1
                    s[2] = 0
            else:
                s[2] = stage + 1
        if not col:
            break
        yield col


def _emit_order(schedule, tiles):
    """Yield (tile_index, eval_desc) in device MASK-CONSUMPTION order (the
    phase4 / stage-3 emissions of the pipeline)."""
    for col in _pipeline_cols(schedule, tiles):
        for (_, q, ev, stage) in col:
            if stage != 3:
                continue
            for t in (2 * q, 2 * q + 1):
                if ev < len(schedule[t]):
                    yield t, schedule[t][ev]


_BUILD_CACHE = {}

LAST_EXEC_NS = None  # set when BASSK_TRACE=1


def _install_ntff_hook():
    """The agent image lacks antenv.axon_hooks; synthesize it so
    run_bass_kernel_spmd(trace=True) can reach the NTFF profiler."""
    import sys
    import types
    if "antenv.axon_hooks" in sys.modules:
        return True
    try:
        import antenv
        from trn_agent_boot.trn_boot import _ntff_profile_via_ctypes
        hook = _ntff_profile_via_ctypes("/opt/axon/libaxon_pjrt.so")
        if hook is None:
            return False
        mod = types.ModuleType("antenv.axon_hooks")
        mod.get_axon_ntff_profile_hook = lambda: hook
        mod.set_axon_ntff_profile_hook = lambda h: None
        sys.modules["antenv.axon_hooks"] = mod
        antenv.axon_hooks = mod
        return True
    except Exception:
        return False


def _build(schedule, use_bias, n_core):
    """schedule: tuple over tiles of tuples of (is_partial, use_mask).

    Masked evals read consecutive NTILE-wide slots of the packed
    per-core mask tensor, in schedule order."""
    tiles = n_core // NTILE
    npair_cols = (tiles // 2) * NTILE          # SBUF columns (pair-major)
    n_masked = sum(1 for tev in schedule for (_, m) in tev if m)
    nm = max(1, n_masked)
    nc = bacc.Bacc("TRN2", target_bir_lowering=False, debug=False,
                   num_devices=NCORES)

    # xm dram rows 0-7: even tiles' 8 x-rows; rows 8-15: odd tiles'.
    xm_d = nc.declare_dram_parameter("xm", [16, npair_cols], F32, isOutput=False)
    mk_d = nc.declare_dram_parameter("maskcat", [8, nm * NTILE], F32,
                                     isOutput=False)
    w1_d = nc.declare_dram_parameter("w1", [8, 512], BF16, isOutput=False)
    w2_d = nc.declare_dram_parameter("w2", [128, 512], BF16, isOutput=False)
    w3_d = nc.declare_dram_parameter("w3", [128, 512], BF16, isOutput=False)
    w4_d = nc.declare_dram_parameter("w4", [128, 8], BF16, isOutput=False)
    if use_bias:
        b123_d = nc.declare_dram_parameter("b123", [128, 6], F32, isOutput=False)
        b4_d = nc.declare_dram_parameter("b4r", [8, 1], F32, isOutput=False)
    # out rows 0-3: even tiles' state; rows 4-7: odd tiles'.
    out_d = nc.declare_dram_parameter("outT", [8, npair_cols], F32, isOutput=True)

    chunks = [(c0, min(c0 + CHUNK_TILES, tiles))
              for c0 in range(0, tiles, CHUNK_TILES)]

    with tile.TileContext(nc) as tc, ExitStack() as ctx:
        const = ctx.enter_context(tc.tile_pool(name="const", bufs=1))
        data = ctx.enter_context(tc.tile_pool(name="data", bufs=1))
        stage = ctx.enter_context(tc.tile_pool(name="stage", bufs=3))
        acts = ctx.enter_context(tc.tile_pool(name="acts", bufs=10))
        tmp = ctx.enter_context(tc.tile_pool(name="tmp", bufs=2))
        ps_h = ctx.enter_context(tc.tile_pool(name="ps_h", bufs=4, space="PSUM"))

        # ---- weights (host-pre-rounded bf16).  w1 is replicated at
        # partitions 0-7 and 64-71 for the two L1 row strips.
        w1 = const.tile([OPAR + 8, 512], BF16)
        nc.gpsimd.dma_start(out=w1[0:8, :], in_=w1_d[:, :])
        nc.gpsimd.dma_start(out=w1[OPAR:OPAR + 8, :], in_=w1_d[:, :])
        w2 = const.tile([128, 512], BF16)
        nc.gpsimd.dma_start(out=w2, in_=w2_d[:, :])
        w3 = const.tile([128, 512], BF16)
        nc.gpsimd.dma_start(out=w3, in_=w3_d[:, :])
        w4 = const.tile([128, 8], BF16)
        nc.gpsimd.dma_start(out=w4, in_=w4_d[:, :])
        if use_bias:
            b123 = const.tile([128, 6], F32)
            nc.gpsimd.dma_start(out=b123, in_=b123_d[:, :])
            b4r = const.tile([OPAR + 4, 1], F32)
            nc.gpsimd.dma_start(out=b4r[0:4, :], in_=b4_d[0:4, :])
            nc.gpsimd.dma_start(out=b4r[OPAR:OPAR + 4, :], in_=b4_d[4:8, :])

        # ---- front-load the one-time silu ACT table load (~2.7us) while the
        # input DMA is still in flight (w1 is the first weight DMA to land)
        dummy = const.tile([8, 64], BF16)
        nc.scalar.activation(dummy, w1[0:8, 0:64], Silu)

        # ---- PE warm-up: back-to-back matmuls on the weight tiles keep the
        # PE busy (HAM stays un-throttled) while the xm DMA lands.
        warm = ps_h.tile([128, 2 * NTILE], F32, tag="h")
        for _ in range(WARMUP_MMS):
            nc.tensor.matmul(warm[:, 0:NTILE], w1[0:8, 0:128],
                             w1[0:8, 0:NTILE], start=True, stop=True)

        # ---- the full per-core dataset stays resident in SBUF; fp32 state
        # rows accumulate at full precision, each eval takes a rounded bf16
        # snapshot for the PE.
        xm = data.tile([OPAR + 8, npair_cols], F32)
        mkc = data.tile([OPAR + 4, nm * NTILE], F32)
        nc.gpsimd.dma_start(out=mkc[0:4, :], in_=mk_d[0:4, :])
        nc.gpsimd.dma_start(out=mkc[OPAR:OPAR + 4, :], in_=mk_d[4:8, :])
        mask_slot = [0]
        for (c0, c1) in chunks:
            cs = bass.ds(c0 * NTILE // 2, (c1 - c0) * NTILE // 2)
            nc.sync.dma_start(out=xm[0:8, cs], in_=xm_d[0:8, cs])
            nc.sync.dma_start(out=xm[OPAR:OPAR + 8, cs], in_=xm_d[8:16, cs])

        def phase1(q, members):
            # L1 for the pair: bf16 snapshot + up to 4 row-strip-concurrent
            # K=8 matmuls; silu per member.
            qs = bass.ds(q * NTILE, NTILE)
            xr = stage.tile([OPAR + 8, NTILE], BF16, tag="x16")
            if len(members) == 2:
                # one DVE op spanning both members' partition homes (the rows
                # in between are unused; copying them is free — DVE cost is
                # free-dim only)
                nc.vector.tensor_copy(xr[0:OPAR + 8, :], xm[0:OPAR + 8, qs])
            else:
                base = members[0][1]
                nc.vector.tensor_copy(xr[base:base + 8, :], xm[base:base + 8, qs])
            hps = []
            for (t, base, is_partial, _) in members:
                w1off = 256 if is_partial else 0
                hp = ps_h.tile([128, 2 * NTILE], F32, tag="h")
                nc.tensor.matmul(hp[:, 0:NTILE],
                                 w1[base:base + 8, w1off:w1off + 128],
                                 xr[base:base + 8, :], start=True, stop=True,
                                 tile_position=(base, 0))
                nc.tensor.matmul(hp[:, NTILE:2 * NTILE],
                                 w1[base:base + 8, w1off + 128:w1off + 256],
                                 xr[base:base + 8, :], start=True, stop=True,
                                 tile_position=(base, 0))
                hps.append(hp)
            out = {}
            for (t, base, is_partial, _), hp in zip(members, hps):
                if use_bias:
                    nc.vector.tensor_scalar_add(hp[:, 0:NTILE], hp[:, 0:NTILE], b123[:, 0:1])
                    nc.vector.tensor_scalar_add(hp[:, NTILE:], hp[:, NTILE:], b123[:, 1:2])
                hs = acts.tile([128, 2 * NTILE], BF16, tag="h")
                nc.scalar.activation(hs, hp, Silu)
                out[t] = hs
            return out

        def phase_mid(t, w, hin, boff):
            # L2/L3: 4 matmuls (K=128 x2 accumulate, M=128 x2), silu
            hp = ps_h.tile([128, 2 * NTILE], F32, tag="h")
            # kt-outer so consecutive matmuls target alternating psum banks
            # (same-bank back-to-back accumulates serialize the array drain)
            for kt in range(2):
                for mt in range(2):
                    nc.tensor.matmul(
                        hp[:, mt * NTILE:(mt + 1) * NTILE],
                        w[:, kt * 256 + mt * 128: kt * 256 + (mt + 1) * 128],
                        hin[:, kt * NTILE:(kt + 1) * NTILE],
                        start=(kt == 0), stop=(kt == 1))
            if use_bias:
                nc.vector.tensor_scalar_add(hp[:, 0:NTILE], hp[:, 0:NTILE], b123[:, boff:boff + 1])
                nc.vector.tensor_scalar_add(hp[:, NTILE:], hp[:, NTILE:], b123[:, boff + 1:boff + 2])
            hs = acts.tile([128, 2 * NTILE], BF16, tag="h")
            nc.scalar.activation(hs, hp, Silu)
            return hp, hs

        def phase4(q, members, h3):
            # L4 for the pair: the member's delta is written back into its own
            # (already-consumed) h3 psum tile, rows base..base+4 — has_written
            # is per-element, so start=True over the stale bank is safe.  The
            # even member's rows 0-3 and the odd member's rows 64-67 go to
            # different column strips (concurrent).
            qs = bass.ds(q * NTILE, NTILE)
            for kt in range(2):
                for (t, base, _, _) in members:
                    hp3, hs3 = h3[t]
                    nc.tensor.matmul(hp3[base:base + 4, 0:NTILE],
                                     w4[:, 4 * kt:4 * kt + 4],
                                     hs3[:, kt * NTILE:(kt + 1) * NTILE],
                                     start=(kt == 0), stop=(kt == 1),
                                     tile_position=(0, base))
            for (t, base, _, use_mask) in members:
                db = h3[t][0][base:base + 4, 0:NTILE]
                if use_bias:
                    nc.vector.tensor_scalar_add(db, db, b4r[base:base + 4, 0:1])
                if not use_mask:
                    nc.vector.tensor_add(xm[base:base + 4, qs],
                                         xm[base:base + 4, qs], db)
                else:
                    j = mask_slot[0]
                    mask_slot[0] += 1
                    dm = tmp.tile([OPAR + 4, NTILE], F32, tag="dm")
                    nc.vector.tensor_mul(dm[base:base + 4, :], db,
                                         mkc[base:base + 4,
                                             j * NTILE:(j + 1) * NTILE])
                    nc.vector.tensor_add(xm[base:base + 4, qs],
                                         xm[base:base + 4, qs],
                                         dm[base:base + 4, :])

        # ---- stage-skewed software-pipeline emission: 4 pair-slots, each at
        # a different layer stage per column, for a uniform PE/ACT mix.
        def members_of(q, ev):
            mem = []
            if ev < len(schedule[2 * q]):
                mem.append((2 * q, 0, *schedule[2 * q][ev]))
            if ev < len(schedule[2 * q + 1]):
                mem.append((2 * q + 1, OPAR, *schedule[2 * q + 1][ev]))
            return mem

        ctxs = [{} for _ in range(NSLOTS)]
        for col in _pipeline_cols(schedule, tiles):
            for (i, q, ev, stg) in col:
                cx = ctxs[i]
                if stg == 0:
                    cx["mem"] = members_of(q, ev)
                    cx["h1"] = phase1(q, cx["mem"])
                elif stg == 1:
                    cx["h2"] = {t: phase_mid(t, w2, cx["h1"][t], 2)[1]
                                for (t, *_) in cx["mem"]}
                elif stg == 2:
                    cx["h3"] = {t: phase_mid(t, w3, cx["h2"][t], 4)
                                for (t, *_) in cx["mem"]}
                else:
                    phase4(q, cx["mem"], cx["h3"])

        # per-pair output DMAs: each fires as soon as its pair's final state
        # update lands (subtile deps), so only the last pair's 2 small DMAs
        # remain in the tail
        for q in range(tiles // 2):
            qs = bass.ds(q * NTILE, NTILE)
            nc.sync.dma_start(out=out_d[0:4, qs], in_=xm[0:4, qs])
            nc.sync.dma_start(out=out_d[4:8, qs], in_=xm[OPAR:OPAR + 4, qs])

    nc.compile()
    return nc


def kernel(state_dz, z_frac, W1, b1, W2, b2, W3, b3, W4, b4):
    global LAST_EXEC_NS
    import os

    state_dz = np.ascontiguousarray(state_dz, dtype=np.float32)
    z_frac = np.ascontiguousarray(z_frac, dtype=np.float32)
    W1 = np.asarray(W1, np.float32); W2 = np.asarray(W2, np.float32)
    W3 = np.asarray(W3, np.float32); W4 = np.asarray(W4, np.float32)
    b1 = np.asarray(b1, np.float32); b2 = np.asarray(b2, np.float32)
    b3 = np.asarray(b3, np.float32); b4 = np.asarray(b4, np.float32)

    B = state_dz.shape[0]
    assert B % (NCORES * 2 * NTILE) == 0, \
        f"B={B} must be divisible by {NCORES * 2 * NTILE}"
    n_core = B // NCORES
    tiles = n_core // NTILE

    # ---- host-side derived quantities (bitwise-identical fp32 ops vs jax)
    dz_sub = (state_dz[:, 5] / np.float32(8.0)).astype(np.float32)
    cont = (z_frac * np.float32(NSTEPS)).astype(np.float32)
    n_full = np.floor(cont).astype(np.float32)
    frac = (cont - n_full).astype(np.float32)
    dz_part = (frac * dz_sub).astype(np.float32)
    has_part = (frac > np.float32(1e-6)).astype(np.float32)
    n_int = np.minimum(n_full, NSTEPS).astype(np.int64)

    # ---- sort desc by n_full, deal round-robin to cores
    order = np.argsort(-n_int, kind="stable")
    perms = [order[c::NCORES] for c in range(NCORES)]

    # ---- build per-core xm arrays [16, npair_cols]: rows 0-7 even tiles,
    # rows 8-15 odd tiles, pair q at columns [q*NTILE:(q+1)*NTILE]
    npair_cols = (tiles // 2) * NTILE
    xms = []
    for c in range(NCORES):
        p = perms[c]
        x8 = np.zeros((8, n_core), np.float32)
        x8[0:4] = state_dz[p, 0:4].T
        x8[R_QOP] = state_dz[p, 4]
        x8[R_DZSUB] = dz_sub[p]
        x8[R_DZPART] = dz_part[p]
        xg = np.zeros((16, npair_cols), np.float32)
        x8t = x8.reshape(8, tiles, NTILE)
        xg = np.concatenate([x8t[:, 0::2, :], x8t[:, 1::2, :]],
                            axis=0).reshape(16, npair_cols)
        xms.append(np.ascontiguousarray(xg))

    # ---- union schedule across cores (SPMD: one program for all cores)
    sched = []
    for t in range(tiles):
        sl = slice(t * NTILE, (t + 1) * NTILE)
        smax, smin = 0, NSTEPS
        anyp, allp = False, True
        for c in range(NCORES):
            nf = n_int[perms[c][sl]]
            smax = max(smax, int(nf.max()))
            smin = min(smin, int(nf.min()))
            hp = has_part[perms[c][sl]]
            anyp = anyp or bool(hp.any())
            allp = allp and bool(hp.all())
        evals = []
        for s in range(min(smax, NSTEPS)):
            evals.append((False, smin <= s))
        if anyp:
            evals.append((True, not allp))
        sched.append(tuple(evals))
    sched = tuple(sched)

    # masked evals in DEVICE EMISSION order; track each tile's step counter
    masked_evals = []
    step_no = [0] * tiles
    for t, (is_partial, use_mask) in _emit_order(sched, tiles):
        s = None if is_partial else step_no[t]
        if not is_partial:
            step_no[t] += 1
        if use_mask:
            masked_evals.append((t, s))

    # ---- packed mask rows, one NTILE slot per masked eval, per core.
    # Dram rows 0-3 serve even tiles, rows 4-7 odd tiles.
    nm = max(1, len(masked_evals))
    maskcats = [np.zeros((8, nm * NTILE), np.float32) for _ in range(NCORES)]
    for j, (t, s) in enumerate(masked_evals):
        sl = slice(t * NTILE, (t + 1) * NTILE)
        r0 = 0 if t % 2 == 0 else 4
        for c in range(NCORES):
            idx = perms[c][sl]
            row = has_part[idx] if s is None else (n_full[idx] > s).astype(np.float32)
            maskcats[c][r0:r0 + 4, j * NTILE:(j + 1) * NTILE] = row[None, :]

    use_bias = bool(np.any(b1) or np.any(b2) or np.any(b3) or np.any(b4))

    key = (sched, use_bias, n_core)
    if key not in _BUILD_CACHE:
        _BUILD_CACHE[key] = _build(sched, use_bias, n_core)
    nc = _BUILD_CACHE[key]

    # ---- weight tensors in lhsT layouts (bf16)
    w1h = np.zeros((8, 512), np.float32)
    w1h[0:6, 0:256] = W1                      # full: state,qop,dz_sub
    w1h[0:5, 256:512] = W1[0:5]               # partial: dz slot zeroed,
    w1h[6, 256:512] = W1[5]                   # dz weight reads dz_partial row
    w1h = w1h.astype(ml_dtypes.bfloat16)
    w2h = np.concatenate([W2[0:128], W2[128:256]], axis=1).astype(ml_dtypes.bfloat16)
    w3h = np.concatenate([W3[0:128], W3[128:256]], axis=1).astype(ml_dtypes.bfloat16)
    w4h = np.concatenate([W4[0:128], W4[128:256]], axis=1).astype(ml_dtypes.bfloat16)

    in_map = {"w1": w1h, "w2": w2h, "w3": w3h, "w4": w4h}
    if use_bias:
        b123 = np.stack([b1[0:128], b1[128:256], b2[0:128], b2[128:256],
                         b3[0:128], b3[128:256]], axis=1).astype(np.float32)
        in_map["b123"] = b123
        in_map["b4r"] = np.concatenate([b4, b4]).reshape(8, 1).astype(np.float32)

    in_maps = [{**in_map, "xm": xms[c], "maskcat": maskcats[c]}
               for c in range(NCORES)]

    trace = os.environ.get("BASSK_TRACE") == "1" and _install_ntff_hook()
    try:
        res = run_bass_kernel_spmd(nc, in_maps, list(range(NCORES)), trace=trace)
    except Exception:
        if not trace:
            raise
        res = run_bass_kernel_spmd(nc, in_maps, list(range(NCORES)), trace=False)
    LAST_EXEC_NS = res.exec_time_ns

    out = np.empty((B, 4), np.float32)
    for c in range(NCORES):
        og = res.results[c]["outT"]                       # [8, npair_cols]
        oc = np.empty((4, n_core), np.float32)
        ocv = oc.reshape(4, tiles, NTILE)
        ogv = og.reshape(8, tiles // 2, NTILE)
        ocv[:, 0::2, :] = ogv[0:4]
        ocv[:, 1::2, :] = ogv[4:8]
        out[perms[c], :] = oc.T
    return out
